# revision 12
# baseline (speedup 1.0000x reference)
"""Trainium2 Bass kernel for the MetricLearning pairwise loss.

Reference math:
    d2[i,j] = max(||x_i||^2 + ||x_j||^2 - 2 x_i.x_j, EPS)
    a = d2/(2k)/sigma^2 ; b = d2/(2k)/omega^2 ; c1 = k/2-1
    per_pair = same ? (-c1*log(a) + a/2) : (c1*log(b) - b/2)
    loss = sum_{i<j} per_pair

Decomposition (L = log d2):
    loss = c1*SUM_{i<j} L                      [DEVICE: main]
         - 2c1*SUM_same L                      [DEVICE: same-label corr.]
         + c1*logB*Npairs - (B/2)*SUM d2       [HOST: exact O(N*D)]
         - c1*(logA+logB)*Nsame + ((A+B)/2)*SUM_same d2   [HOST]
    using SUM_{i<j} x_i.x_j = (||SUM x||^2 - SUM ||x||^2)/2 per group.

So the device computes ONLY log-sums: fp8 DoubleRow gram -> aug matmul
adds -sq_j/2 -> ACT Ln(-2t + sq_i) with accum_out (row sums). Rows are
globally SORTED BY LABEL; same-label pairs live within a block or in the
128-wide corner between consecutive blocks, handled by HOST-built 0/1
masks (strict-upper same-label) via fused scalar_tensor_tensor ops.
Within-block triangles: the PSUM tile is pre-multiplied by a strict-upper
mask; masked cells become Ln(sq_i), a host-known constant subtracted off.

Layout: a linearized "conveyor" of the per-core 8704 gram columns in
weight-coherent run order [ls0u0|ls1u0|ls0u1|ls1u1], cut into 2048-col
PSUM supertiles (4 banks, pool bufs=2). kc2-outer emission inside each
run*tile segment maximizes stationary-weight reuse. Output is the raw
[128, 18] partial-sum accumulator; the final coefficient dot happens on
host in f64.

Sharding: 16 row-blocks of 256; the K16 block-pair graph is oriented so
every core owns one even block (8 partners) + one odd block (7 partners)
plus both within-block triangles -> identical SPMD program on all 8
cores, per-core variation only in input data (slab permutation).
"""

import numpy as np
import ml_dtypes

N = 4096
D = 1024
P = 128
NB = 16          # row blocks
BLK = 256        # rows per block
KC = D // P      # k chunks (8)
NCORES = 8

SIGMA = 0.2
OMEGA = 1.0
K_F = float(N)
C1 = K_F / 2.0 - 1.0                      # 2047
A_C = 1.0 / (2.0 * K_F * SIGMA * SIGMA)   # 1/327.68
B_C = 1.0 / (2.0 * K_F * OMEGA * OMEGA)   # 1/8192
LOG_A = float(np.log(A_C))
LOG_B = float(np.log(B_C))
CORNER_W = 128

NACC = 18        # 12 main-L cols + 4 diag-ML + 2 corner-ML
MSK_W = 1792     # 2*256 upper + 4*256 same + 2*128 corner

# ---- conveyor structure -----------------------------------------------
# runs in emission order: (ls, u, atom slot list). ls-outer so the second
# half of the conveyor (slabs 9-15) tolerates late DMA arrival.
RUNS = [(0, 0, list(range(0, 9))),
        (0, 1, list(range(0, 9))),
        (1, 0, [1] + list(range(9, 16))),
        (1, 1, [1] + list(range(9, 16)))]
TILE_W = 2048


def _build_conveyor():
    """Greedy chunking of the global column conveyor.

    chunk = (gcol, s0, ns): ns<=2 consecutive slots, 512-aligned bank fit.
    segment = chunks of one run within one psum tile.
    Returns (segments, tiles) where
      segments: list of dicts {tile, ls, u, chunks:[(local_lo, s0, ns)]}
      tiles: list of (tile_idx, width)
    """
    g = 0
    segments = []
    for ls, u, atoms in RUNS:
        i = 0
        cur = None
        while i < len(atoms):
            s0 = atoms[i]
            ns = 1
            if (g % 512 == 0 and i + 1 < len(atoms)
                    and atoms[i + 1] == s0 + 1):
                ns = 2
            t = g // TILE_W
            assert (g + 256 * ns - 1) // TILE_W == t  # chunk within tile
            if cur is None or cur["tile"] != t:
                cur = {"tile": t, "ls": ls, "u": u, "chunks": []}
                segments.append(cur)
            cur["chunks"].append((g - t * TILE_W, s0, ns))
            g += 256 * ns
            i += ns
        # force new segment at run end
        cur = None
    assert g == 8704
    ntiles = (g + TILE_W - 1) // TILE_W
    tiles = [(t, min(TILE_W, g - t * TILE_W)) for t in range(ntiles)]
    return segments, tiles


SEGMENTS, TILES = _build_conveyor()

# consumers per tile: list of specs
#  ("X", lo, w, g, cmain)                     plain cross ACT
#  ("XC", lo, w, g, cmain, cidx)              cross ACT + corner ML
#  ("D", lo, g, u, same_idx, cmain, cml)      masked diag ACT + same ML
# g = 2*ls + u indexes rowd column. Built to match SEGMENTS above.
CONSUMERS = {
    0: [("D", 0, 0, 0, 0, 0, 12), ("X", 256, 1792, 0, 1)],
    1: [("X", 0, 256, 0, 2), ("D", 256, 1, 1, 1, 3, 13),
        ("XC", 512, 1536, 1, 4, 0)],
    2: [("X", 0, 512, 1, 5), ("D", 512, 2, 0, 2, 6, 14),
        ("X", 768, 1280, 2, 7)],
    3: [("X", 0, 512, 2, 8), ("D", 512, 3, 1, 3, 9, 15),
        ("XC", 768, 1280, 3, 10, 1)],
    4: [("X", 0, 512, 3, 11)],
}
# msk column layout
MSK_UP = {0: 0, 1: 256}                 # strict upper per u
MSK_SAME = {g: 512 + 256 * g for g in range(4)}   # keyed by g = 2*ls+u
MSK_CORNER = {0: 1536, 1: 1664}


def _partners(d):
    """Block orientation: edge {i,j} (i<j) owned by i if i+j odd else j."""
    l0, l1 = 2 * d, 2 * d + 1
    p8 = [j for j in range(l0 + 1, NB) if j % 2 == 1] + \
         [i for i in range(0, l0) if i % 2 == 0]
    p7 = [j for j in range(l1 + 1, NB) if j % 2 == 0] + \
         [i for i in range(0, l1) if i % 2 == 1]
    assert len(p8) == 8 and len(p7) == 7 and l1 in p8
    return l0, l1, p8, p7


def _core_slabs(d):
    """Slot -> block id (16 slots). slot0=own even, slot1=own odd, and
    slot9 (first partner of the odd block) pinned to block 2d+2 when it
    exists so the consecutive-pair corner lands at a fixed slot."""
    l0, l1, p8, p7 = _partners(d)
    rest8 = [p for p in p8 if p != l1]
    nxt = l1 + 1
    if nxt in p7:
        p7 = [nxt] + [p for p in p7 if p != nxt]
    slabs = [l0, l1] + rest8 + list(p7)
    assert len(slabs) == NB and len(set(slabs)) == NB
    return slabs


_PROG_CACHE = {}


def _build_program():
    if "nc" in _PROG_CACHE:
        return _PROG_CACHE["nc"]
    import concourse.bass as bass  # noqa: F401
    import concourse.bacc as bacc
    import concourse.mybir as mybir
    import concourse.tile as tile

    F32 = mybir.dt.float32
    BF16 = mybir.dt.bfloat16
    FP8 = mybir.dt.float8e4
    AF = mybir.ActivationFunctionType
    ALU = mybir.AluOpType
    DR = mybir.MatmulPerfMode.DoubleRow

    nc = bacc.Bacc("TRN2", target_bir_lowering=False, debug=False,
                   num_devices=NCORES)
    xtp_d = nc.dram_tensor("xtp", [NB, P, KC, BLK], FP8,
                           kind="ExternalInput").ap()
    aug_d = nc.dram_tensor("aug", [2, N], BF16, kind="ExternalInput").ap()
    msk_d = nc.dram_tensor("msk", [P, MSK_W], BF16,
                           kind="ExternalInput").ap()
    rowd_d = nc.dram_tensor("rowd", [P, 4], F32, kind="ExternalInput").ap()
    out_d = nc.dram_tensor("out", [P, NACC], F32, kind="ExternalOutput").ap()

    with tile.TileContext(nc) as tc:
        with (
            tc.tile_pool(name="persist", bufs=1) as persist,
            tc.tile_pool(name="lt", bufs=2) as ltp,
            tc.tile_pool(name="t2", bufs=2) as t2p,
            tc.tile_pool(name="psum", bufs=2, space="PSUM") as psum,
        ):
            xall = persist.tile([P, NB, KC, BLK], FP8, tag="xall")
            msk = persist.tile([P, MSK_W], BF16, tag="msk")
            augs = persist.tile([2, N], BF16, tag="augs")
            rowd = persist.tile([P, 4], F32, tag="rowd")
            ones2 = persist.tile([2, P], BF16, tag="ones2")
            wrhs = persist.tile([2, 512], BF16, tag="wrhs")
            acc = persist.tile([P, NACC], F32, tag="acc")
            dump = persist.tile([P, 1792], BF16, tag="dump")
            gdump = persist.tile([P, 256], BF16, tag="gdump")
            tiny = persist.tile([1, 1], F32, tag="tiny")

            nc.gpsimd.memset(ones2[:], 1.0)
            nc.gpsimd.memset(wrhs[:], 1.0)
            # preload the natural_log table set during the DMA head
            nc.scalar.activation(tiny[:], ones2[0:1, 0:1], AF.Ln)

            nc.scalar.dma_start(out=augs[:], in_=aug_d[:])
            nc.scalar.dma_start(out=rowd[:], in_=rowd_d[:])
            nc.scalar.dma_start(out=msk[:], in_=msk_d[:])
            # single queue: serial transfers arrive slab-by-slab in the
            # order the conveyor consumes them (parallel queues interleave
            # packets and delay every completion to the end)
            for s in range(NB):
                nc.sync.dma_start(out=xall[:, s], in_=xtp_d[s])

            def lhsT(ls, u, k2):
                return xall[:, ls, 2 * k2:2 * k2 + 2, 128 * u:128 * (u + 1)]

            def rhs(s0, ns, k2):
                return xall[:, s0:s0 + ns, 2 * k2:2 * k2 + 2, :] \
                    .rearrange("p s k b -> p k s b")

            tiles = {}

            def get_tile(t):
                if t not in tiles:
                    tiles[t] = psum.tile([P, TILE_W], F32,
                                         name=f"T{t}", tag="T")
                return tiles[t]

            # warmup: keep PE busy during the DMA head, flips HAM early
            T0 = get_tile(0)
            for _ in range(4):
                nc.tensor.matmul(T0[:, 0:512], ones2[:, :], wrhs[:, :],
                                 start=True, stop=True)

            def emit_consumers(t):
                T = tiles[t]
                for spec in CONSUMERS[t]:
                    kind = spec[0]
                    if kind == "D":
                        _, lo, g, u, sk, cmain, cml = spec
                        t2 = t2p.tile([P, 256], F32, tag="t2")
                        nc.vector.scalar_tensor_tensor(
                            out=t2[:], in0=T[:, lo:lo + 256], scalar=1.0,
                            in1=msk[:, MSK_UP[u]:MSK_UP[u] + 256],
                            op0=ALU.mult, op1=ALU.mult)
                        ltd = ltp.tile([P, 256], BF16, tag="ltd")
                        nc.scalar.activation(
                            ltd[:], t2[:], AF.Ln,
                            bias=rowd[:, g:g + 1], scale=-2.0,
                            accum_out=acc[:, cmain:cmain + 1])
                        ms = MSK_SAME[sk]
                        nc.vector.scalar_tensor_tensor(
                            out=gdump[:, 0:256], in0=ltd[:], scalar=1.0,
                            in1=msk[:, ms:ms + 256],
                            op0=ALU.mult, op1=ALU.mult,
                            accum_out=acc[:, cml:cml + 1])
                    elif kind == "X":
                        _, lo, w, g, cmain = spec
                        nc.scalar.activation(
                            dump[:, 0:w], T[:, lo:lo + w], AF.Ln,
                            bias=rowd[:, g:g + 1], scale=-2.0,
                            accum_out=acc[:, cmain:cmain + 1])
                    else:  # XC
                        _, lo, w, g, cmain, ck = spec
                        lt = ltp.tile([P, w], BF16, tag="lt")
                        nc.scalar.activation(
                            lt[:], T[:, lo:lo + w], AF.Ln,
                            bias=rowd[:, g:g + 1], scale=-2.0,
                            accum_out=acc[:, cmain:cmain + 1])
                        mc = MSK_CORNER[ck]
                        nc.vector.scalar_tensor_tensor(
                            out=gdump[:, 0:128], in0=lt[:, 0:128],
                            scalar=1.0, in1=msk[:, mc:mc + 128],
                            op0=ALU.mult, op1=ALU.mult,
                            accum_out=acc[:, 16 + ck:17 + ck])

            for si, seg in enumerate(SEGMENTS):
                t = seg["tile"]
                ls, u = seg["ls"], seg["u"]
                T = get_tile(t)
                # chunk-outer: consume slabs in DMA arrival order; the
                # per-MM LDWEIGHTS is hidden behind the 512-col stream
                for lo, s0, ns in seg["chunks"]:
                    w = 256 * ns
                    for k2 in range(KC // 2):
                        nc.tensor.matmul(
                            T[:, lo:lo + w], lhsT(ls, u, k2),
                            rhs(s0, ns, k2),
                            start=(k2 == 0), stop=False, perf_mode=DR)
                    nc.tensor.matmul(T[:, lo:lo + w], ones2[:, :],
                                     augs[:, s0 * 256:s0 * 256 + w],
                                     start=False, stop=True)
                # consumers once this tile's last segment is done
                if si + 1 == len(SEGMENTS) or \
                        SEGMENTS[si + 1]["tile"] != t:
                    emit_consumers(t)

            nc.sync.dma_start(out=out_d[:], in_=acc[:])

    nc.compile()
    _PROG_CACHE["nc"] = nc
    return nc


def _host_prep(outputs, labels):
    """Sort rows by label, build per-core inputs + host-side loss terms."""
    x = np.asarray(outputs, dtype=np.float32)
    lab = np.asarray(labels)
    assert x.shape == (N, D)
    perm = np.argsort(lab, kind="stable")
    xp = x[perm]
    labp = lab[perm].astype(np.int64)

    # label runs (sorted labels)
    change = np.nonzero(np.diff(labp))[0] + 1
    starts = np.concatenate([[0], change])
    ends = np.concatenate([change, [N]])
    ngs = ends - starts
    max_run = int(ngs.max())
    assert max_run <= CORNER_W, f"label run {max_run} exceeds corner width"

    xq8 = xp.astype(ml_dtypes.float8_e4m3)
    xq = xq8.astype(np.float64)
    # True (unquantized) norms make d2 = sq_i + sq_j - 2*xq_i.xq_j unbiased
    sq = (xp.astype(np.float64) ** 2).sum(axis=1)
    q = (xq ** 2).sum(axis=1)

    # ---- host-exact t/d2 sums (O(N*D)) --------------------------------
    S = xq.sum(axis=0)
    npairs = N * (N - 1) // 2
    sum_d2_all = (N - 1) * sq.sum() - (S @ S - q.sum())
    Sg = np.add.reduceat(xq, starts, axis=0)
    sq_g = np.add.reduceat(sq, starts)
    q_g = np.add.reduceat(q, starts)
    sum_d2_same = float((((ngs - 1) * sq_g) - ((Sg * Sg).sum(axis=1) - q_g)).sum())
    nsame = int((ngs * (ngs - 1) // 2).sum())

    host_t = (C1 * LOG_B * npairs - 0.5 * B_C * sum_d2_all
              - C1 * (LOG_A + LOG_B) * nsame
              + 0.5 * (A_C + B_C) * sum_d2_same)

    # masked diag cells contribute Ln(sq_i) to the device main accum:
    # per sorted row i, (i%256 + 1) such cells
    sq32 = sq.astype(np.float32).astype(np.float64)
    fillsub = float(((np.arange(N) % BLK + 1) * np.log(sq32)).sum())
    host_add = host_t - C1 * fillsub

    xt_q = np.ascontiguousarray(xq8.T)                               # [D, N]
    neg_half = -0.5 * sq
    hi = neg_half.astype(ml_dtypes.bfloat16)
    lo = (neg_half - hi.astype(np.float64)).astype(ml_dtypes.bfloat16)

    in_maps = []
    for d in range(NCORES):
        slabs = _core_slabs(d)
        cols = np.concatenate(
            [np.arange(b * BLK, (b + 1) * BLK) for b in slabs])
        xtp = np.ascontiguousarray(
            xt_q[:, cols].reshape(KC, P, NB, BLK).transpose(2, 1, 0, 3))
        aug = np.stack([hi[cols], lo[cols]])                       # [2, N]

        rowd = np.zeros((P, 4), dtype=np.float32)
        rows = {}
        for ls in (0, 1):
            for u in (0, 1):
                r = slabs[ls] * BLK + 128 * u + np.arange(P)
                rows[(ls, u)] = r
                rowd[:, 2 * ls + u] = sq32[r]

        msk = np.zeros((P, MSK_W), dtype=np.float64)
        ii = np.arange(P)[:, None]
        jj = np.arange(BLK)[None, :]
        for u in (0, 1):
            msk[:, MSK_UP[u]:MSK_UP[u] + BLK] = (jj > ii + 128 * u)
        for ls in (0, 1):
            for u in (0, 1):
                g = 2 * ls + u
                r = rows[(ls, u)]
                c = slabs[ls] * BLK + np.arange(BLK)
                same = labp[r][:, None] == labp[c][None, :]
                msk[:, MSK_SAME[g]:MSK_SAME[g] + BLK] = \
                    same & (jj > ii + 128 * u)
        for ck, (rls, cslot) in enumerate(((0, 1), (1, 9))):
            r = rows[(rls, 1)]
            c = slabs[cslot] * BLK + np.arange(CORNER_W)
            msk[:, MSK_CORNER[ck]:MSK_CORNER[ck] + CORNER_W] = \
                labp[r][:, None] == labp[c][None, :]

        in_maps.append({
            "xtp": xtp,
            "aug": np.ascontiguousarray(aug),
            "msk": msk.astype(ml_dtypes.bfloat16),
            "rowd": rowd,
        })
    return in_maps, host_add


def _combine(results, host_add):
    total = np.float64(host_add)
    for r in results:
        o = np.asarray(r["out"]).astype(np.float64)
        total += C1 * o[:, :12].sum() - 2.0 * C1 * o[:, 12:].sum()
    return np.asarray(total, dtype=np.float32)


def kernel(**inputs):
    from concourse.bass_utils import run_bass_kernel_spmd
    nc = _build_program()
    in_maps, host_add = _host_prep(inputs["outputs"], inputs["labels"])
    res = run_bass_kernel_spmd(nc, in_maps, core_ids=list(range(NCORES)))
    return _combine(res.results, host_add)
